# revision 16
# baseline (speedup 1.0000x reference)
"""Trainium2 Bass kernel for location-aware attention (AttLoc).

Computation (per batch row b):
  pre_enc = enc @ W_enc + b_enc                  (T, A)
  att_c   = conv1d(att_prev) @ W_att             (T, A)   [folded: M = conv_w.T @ W_att]
  dec_t   = dec_z @ W_dec                        (A,)     [host-computed, per-partition bias]
  e       = tanh(pre_enc + att_c + dec_t) @ W_g + b_g
  w       = softmax(2 * e, masked t >= len)
  c       = sum_t w[t] * enc[t, :]

Distribution: data-parallel over batch B=32 across 8 NeuronCores (4 rows each).
Device data: enc transposed to [e-partition, t-free] bf16 (single HBM read per
row).  The big matmul runs on TensorE in bf16; the context reduction runs on
VectorE against a broadcast copy of w; softmax runs on VectorE/ScalarE.
"""

import numpy as np
import ml_dtypes

# ---------------------------------------------------------------------------
# problem constants (hardcoded per contract — no reading of spec.json)
B, T, E, A = 32, 2048, 1024, 512
N_CORES = 8
BPC = B // N_CORES            # batch rows per core
KF = 31                       # conv taps (2*15+1)
SCALING = 2.0
NEG = -1.0e30
PAD_T = T + 32                # padded att_prev row (15 left, 17 right)

_BF = ml_dtypes.bfloat16


# ---------------------------------------------------------------------------
def _apply_walrus_single_wait_patch():
    """The external walrus build in this container rejects any instruction
    carrying more than one sem-wait command.  Split multi-wait instructions by
    hoisting extra waits onto same-engine InstNoOp's."""
    import concourse.mybir as mybir
    import concourse.tile as tile
    from concourse.vector_clock import ScopedClock

    if getattr(tile.TileContext, "_single_wait_patched", False):
        return

    def _split_waits(tc, inst):
        si = getattr(inst, "sync_info", None)
        if si is None or inst.engine is None:
            return
        waits = list(si.on_wait)
        if len(waits) <= 1:
            return
        for w in waits[:-1]:
            nop = mybir.InstNoOp(
                name=tc.nc.get_next_instruction_name(),
                engine=inst.engine,
                sync_info=mybir.SyncInfo(on_wait=[w], on_update=[]),
                bass_nofuse=True,
            )
            tc._commit_instruction(nop)
        si.on_wait = waits[-1:]

    _orig_cl = tile.TileContext._commit_and_lower

    def _commit_and_lower(self, inst, original_block, old_bb_map, bb_to_exit_bb):
        _split_waits(self, inst)
        return _orig_cl(self, inst, original_block, old_bb_map, bb_to_exit_bb)

    def _drain_and_barrier(self, tick_clock, wait_clock):
        drain_inst = self.nc.sync.drain()
        wait_clock.add_sem_waits(
            drain_inst.ins, ScopedClock({None: tick_clock.global_clock})
        )
        si = drain_inst.ins.sync_info
        waits = list(si.on_wait)
        if len(waits) > 1:
            si.on_wait = waits[:1]
            for w in waits[1:]:
                nop = self.nc.sync.nop(nofuse=True, hint="drain_wait_spill")
                nop.ins.sync_info = mybir.SyncInfo(on_wait=[w], on_update=[])
        self.nc.all_engine_barrier()
        assert self.sems is not None
        popped = self.nc._tile_sem_poison_stack.pop()
        assert popped is self._sem_poison
        self.nc.clear_and_free_semaphores(list(self.sems.allocated().values()))
        self.nc.all_engine_barrier()

    tile.TileContext._commit_and_lower = _commit_and_lower
    tile.TileContext._drain_and_barrier = _drain_and_barrier
    tile.TileContext._single_wait_patched = True


# ---------------------------------------------------------------------------
def _build_program(repeat=1, phase="full"):
    import concourse.bass as bass
    import concourse.mybir as mybir
    import concourse.tile as tile

    _apply_walrus_single_wait_patch()

    F = mybir.dt.float32
    BF = mybir.dt.bfloat16
    AF = mybir.ActivationFunctionType
    ALU = mybir.AluOpType
    AX = mybir.AxisListType

    nc = bass.Bass()
    p_encT = nc.declare_dram_parameter("encT", [BPC, 128, 8, T], BF, isOutput=False)
    p_encN = nc.declare_dram_parameter("encN", [BPC, 128, 16, E], BF, isOutput=False)
    p_appad = nc.declare_dram_parameter("appad", [BPC, PAD_T], BF, isOutput=False)
    p_wenc = nc.declare_dram_parameter("wenc", [128, 8, A], BF, isOutput=False)
    p_ms = nc.declare_dram_parameter("msmall", [32, A], BF, isOutput=False)
    p_wg = nc.declare_dram_parameter("wg2", [128, 4], BF, isOutput=False)
    p_bi = nc.declare_dram_parameter("biasc", [128, BPC * 4], F, isOutput=False)
    p_nm = nc.declare_dram_parameter("negm", [BPC, T], F, isOutput=False)
    p_wout = nc.declare_dram_parameter("w_out", [BPC, T], F, isOutput=True)
    p_cout = nc.declare_dram_parameter("c_out", [BPC, E], F, isOutput=True)

    with tile.TileContext(nc) as tc:
        with (
            tc.tile_pool(name="const", bufs=1) as const,
            tc.tile_pool(name="et", bufs=2) as et_pool,
            tc.tile_pool(name="th", bufs=2) as th_pool,
            tc.tile_pool(name="win", bufs=2) as win_pool,
            tc.tile_pool(name="rows", bufs=2) as row_pool,
            tc.tile_pool(name="en", bufs=1) as en_pool,
            tc.tile_pool(name="ct", bufs=2) as ct_pool,
            tc.tile_pool(name="ppool", bufs=4, space="PSUM") as ppool,
            tc.tile_pool(name="epool", bufs=2, space="PSUM") as epool,
            tc.tile_pool(name="cpool", bufs=2, space="PSUM") as cpool,
            tc.tile_pool(name="dram", bufs=1, space="DRAM") as dram,
        ):
            WE = const.tile([128, 8, A], BF, tag="WE")
            ONE1 = const.tile([1, 1], F, tag="ONE1")
            nc.vector.memset(ONE1[:], 1.0)
            MS4 = const.tile([128, A], BF, tag="MS4")
            WG = const.tile([128, 4], BF, tag="WG")
            BI = const.tile([128, BPC * 4], F, tag="BI")
            nc.sync.dma_start(WE[:], p_wenc[:, :, :])
            for g in range(4):
                nc.sync.dma_start(MS4[32 * g : 32 * g + 32, :], p_ms[:, :])
            nc.sync.dma_start(WG[:], p_wg[:, :])
            nc.sync.dma_start(BI[:], p_bi[:, :])

            iters = [bb % BPC for bb in range(BPC * repeat)]

            def emit_loads(b):
                """Queue batch b's input DMAs. Big streams go on the ACT
                HWDGE ring, small ones on the SP ring."""
                WIN4 = win_pool.tile([128, T], BF, tag="WIN4")
                for g in range(4):
                    nc.sync.dma_start(
                        WIN4[32 * g : 32 * g + 32, :],
                        bass.AP(p_appad, b * PAD_T, [[1, 32], [1, T]]),
                    )
                nm = row_pool.tile([1, T], F, tag="nm")
                nc.sync.dma_start(nm[0:1, :], p_nm[b : b + 1, :])
                ET = et_pool.tile([128, 8, T], BF, tag="ET")
                for k in range(8):
                    nc.sync.dma_start(ET[:, k, :], p_encT[b, :, k, :])
                EN = en_pool.tile([128, 16, E], BF, tag="EN")
                nc.gpsimd.dma_start(EN[:], p_encN[b, :, :, :])
                return WIN4, nm, ET, EN

            loads = emit_loads(iters[0])
            pending_ctx = None
            for bi, b in enumerate(iters):
                WIN4, nm, ET, EN = loads

                # ---- energies: k-major quarters; att starts each PSUM tile
                TH = th_pool.tile([128, 4, T], BF, tag="TH")
                for ih in range(2):
                    for jh in range(2):
                        ii = (2 * ih, 2 * ih + 1)
                        jj = (2 * jh, 2 * jh + 1)
                        Pq = {}
                        for i in ii:
                            for j in jj:
                                P = ppool.tile([128, 512], F, tag="P")
                                Pq[(i, j)] = P
                        for k in range(8):
                            for i in ii:
                                for j in jj:
                                    nc.tensor.matmul(
                                        Pq[(i, j)][:, :],
                                        WE[:, k, i * 128 : (i + 1) * 128],
                                        ET[:, k, j * 512 : (j + 1) * 512],
                                        start=(k == 0),
                                        stop=False,
                                    )
                        for i in ii:
                            for j in jj:
                                nc.tensor.matmul(
                                    Pq[(i, j)][:, :],
                                    MS4[32 * i : 32 * i + 32, i * 128 : (i + 1) * 128],
                                    WIN4[32 * i : 32 * i + 32, j * 512 : (j + 1) * 512],
                                    start=False,
                                    stop=True,
                                    tile_position=(32 * i, 0),
                                )
                        for i in ii:
                            for j in jj:
                                nc.scalar.activation(
                                    TH[:, i, j * 512 : (j + 1) * 512],
                                    Pq[(i, j)][:, :],
                                    AF.Tanh,
                                    bias=BI[:, b * 4 + i : b * 4 + i + 1],
                                    scale=1.0,
                                )

                # ---- queue next iteration's loads before any blocking op --
                if bi + 1 < len(iters):
                    loads = emit_loads(iters[bi + 1])
                if phase == "energy":
                    nc.gpsimd.dma_start(p_wout[b : b + 1, 0:512], TH[0:1, 0, 0:512])
                    continue

                # ---- e row: 2*W_g contraction over a; mask folded in ------
                e_sb = row_pool.tile([1, T], F, tag="e_sb")
                msc = row_pool.tile([1, 8], F, tag="msc")
                pex = row_pool.tile([1, T], F, tag="pex")
                for j in range(4):
                    EP = epool.tile([128, 512], F, tag="EP")
                    for i in range(4):
                        nc.tensor.matmul(
                            EP[0:1, :],
                            WG[:, i : i + 1],
                            TH[:, i, j * 512 : (j + 1) * 512],
                            start=(i == 0),
                            stop=(i == 3),
                        )
                    ej = e_sb[0:1, j * 512 : (j + 1) * 512]
                    nc.vector.tensor_tensor(
                        ej, nm[0:1, j * 512 : (j + 1) * 512], EP[0:1, :], op=ALU.add
                    )
                    nc.vector.tensor_reduce(
                        msc[0:1, 4 + j : 5 + j], ej, axis=AX.X, op=ALU.max
                    )

                # ---- masked softmax ---------------------------------------
                nc.vector.tensor_reduce(msc[0:1, 0:1], msc[0:1, 4:8], axis=AX.X, op=ALU.max)
                nc.scalar.mul(msc[0:1, 1:2], msc[0:1, 0:1], -1.0)
                nc.scalar.activation(
                    pex[0:1, :], e_sb[0:1, :], AF.Exp,
                    bias=msc[0:1, 1:2], scale=1.0, accum_out=msc[0:1, 2:3],
                )
                if phase == "noctx":
                    nc.vector.reciprocal(msc[0:1, 3:4], msc[0:1, 2:3])
                    nc.vector.tensor_scalar_mul(pex[0:1, :], pex[0:1, :], msc[0:1, 3:4])
                    nc.gpsimd.dma_start(p_wout[b : b + 1, :], pex[0:1, :])
                    continue

                # ---- context (deferred one iteration: runs under the next
                # batch's energy phase so PE never blocks on softmax) -------
                def emit_ctx(b, EN, pex):
                    WTP = cpool.tile([128, 512], F, tag="CP")
                    for t_i in range(16):
                        nc.tensor.matmul(
                            WTP[:, t_i : t_i + 1],
                            pex[0:1, 128 * t_i : 128 * (t_i + 1)],
                            ONE1[0:1, 0:1],
                            start=True,
                            stop=True,
                        )
                    wT = ct_pool.tile([128, 16], BF, tag="wT")
                    nc.vector.tensor_copy(wT[:, :], WTP[:, 0:16])
                    cS = ct_pool.tile([1, E], F, tag="cS")
                    for h in range(2):
                        CP = cpool.tile([128, 512], F, tag="CP")
                        for t_i in range(16):
                            g = t_i % 2
                            nc.tensor.matmul(
                                CP[32 * g : 32 * g + 1, :],
                                wT[:, t_i : t_i + 1],
                                EN[:, t_i, h * 512 : (h + 1) * 512],
                                start=(t_i < 2),
                                stop=(t_i >= 14),
                                tile_position=(0, 32 * g),
                            )
                        half = cS[0:1, h * 512 : (h + 1) * 512]
                        nc.vector.tensor_copy(half, CP[0:1, :])
                        nc.vector.tensor_tensor(half, half, CP[32:33, :], op=ALU.add)
                    nc.gpsimd.dma_start(p_cout[b : b + 1, :], cS[0:1, :])

                if pending_ctx is not None:
                    emit_ctx(*pending_ctx)
                pending_ctx = (b, EN, pex)

                # softmax tail after the deferred ctx so the DVE stream
                # doesn't block the previous batch's context ops
                nc.vector.reciprocal(msc[0:1, 3:4], msc[0:1, 2:3])
                nc.vector.tensor_scalar_mul(pex[0:1, :], pex[0:1, :], msc[0:1, 3:4])
                nc.gpsimd.dma_start(p_wout[b : b + 1, :], pex[0:1, :])

            if pending_ctx is not None:
                emit_ctx(*pending_ctx)

    return nc


# ---------------------------------------------------------------------------
def kernel(enc_hs_pad, enc_hs_len, dec_z, att_prev,
           W_enc, b_enc, W_dec, W_att, conv_w, W_g, b_g):
    from concourse.bass_utils import run_bass_kernel_spmd

    enc = np.ascontiguousarray(np.asarray(enc_hs_pad, dtype=np.float32))
    lens = np.asarray(enc_hs_len).astype(np.int64)
    dec_z = np.asarray(dec_z, dtype=np.float32)
    att_prev = np.asarray(att_prev, dtype=np.float32)
    W_enc = np.asarray(W_enc, dtype=np.float32)
    b_enc = np.asarray(b_enc, dtype=np.float32)
    W_dec = np.asarray(W_dec, dtype=np.float32)
    W_att = np.asarray(W_att, dtype=np.float32)
    conv_w = np.asarray(conv_w, dtype=np.float32)
    W_g = np.asarray(W_g, dtype=np.float32)
    b_g = np.asarray(b_g, dtype=np.float32)

    # ---- host-side prep (tiny reshapes/folds) -----------------------------
    # enc transposed: [b, p, c, t] = enc[b, t, c*128 + p]
    encT = enc.reshape(B, T, 8, 128).transpose(0, 3, 2, 1).astype(_BF)
    encN = enc.reshape(B, 16, 128, E).transpose(0, 2, 1, 3).astype(_BF)
    appad = np.zeros((B, PAD_T), dtype=_BF)
    appad[:, 15 : 15 + T] = att_prev.astype(_BF)
    # wenc: [p, k, a] = W_enc[k*128 + p, a]
    wenc = W_enc.reshape(8, 128, A).transpose(1, 0, 2).astype(_BF)
    # folded location-conv: att_c[t, a] = sum_k M[k, a] * appad[t + k]
    Mfold = conv_w[:, 0, :].T @ W_att            # (31, A)
    msmall = np.zeros((32, A), dtype=_BF)
    msmall[:KF] = Mfold.astype(_BF)
    # wg2: [p, i] = SCALING * W_g[i*128 + p]
    wg2 = (SCALING * W_g[:, 0]).reshape(4, 128).T.astype(_BF)
    wg2 = np.ascontiguousarray(wg2)
    # per-partition tanh bias: dec proj + b_enc
    bias = dec_z @ W_dec + b_enc                 # (B, A)
    # neg mask rows (scaled b_g folded in): 2*b_g where valid else -1e30
    tt = np.arange(T)[None, :]
    negm = np.where(tt < lens[:, None], SCALING * b_g[0], NEG).astype(np.float32)

    nc = _build_program()

    in_maps = []
    for i in range(N_CORES):
        sl = slice(i * BPC, (i + 1) * BPC)
        bias_core = (
            bias[sl].reshape(BPC, 4, 128).transpose(2, 0, 1).reshape(128, BPC * 4)
        )
        in_maps.append({
            "encT": np.ascontiguousarray(encT[sl]),
            "encN": np.ascontiguousarray(encN[sl]),
            "appad": np.ascontiguousarray(appad[sl]),
            "wenc": np.ascontiguousarray(wenc),
            "msmall": msmall,
            "wg2": wg2,
            "biasc": np.ascontiguousarray(bias_core),
            "negm": np.ascontiguousarray(negm[sl]),
        })

    res = run_bass_kernel_spmd(nc, in_maps, list(range(N_CORES)))

    w_full = np.concatenate([res.results[i]["w_out"] for i in range(N_CORES)], axis=0)
    c_full = np.concatenate(
        [res.results[i]["c_out"] for i in range(N_CORES)], axis=0
    )
    return c_full.astype(np.float32), w_full.astype(np.float32)


# revision 17
# speedup vs baseline: 1.0927x; 1.0927x over previous
"""Trainium2 Bass kernel for location-aware attention (AttLoc).

Computation (per batch row b):
  pre_enc = enc @ W_enc + b_enc                  (T, A)
  att_c   = conv1d(att_prev) @ W_att             (T, A)   [folded: M = conv_w.T @ W_att]
  dec_t   = dec_z @ W_dec                        (A,)     [host-computed, per-partition bias]
  e       = tanh(pre_enc + att_c + dec_t) @ W_g + b_g
  w       = softmax(2 * e, masked t >= len)
  c       = sum_t w[t] * enc[t, :]

Distribution: data-parallel over batch B=32 across 8 NeuronCores (4 rows each).
Device data: enc transposed to [e-partition, t-free] bf16 (single HBM read per
row).  The big matmul runs on TensorE in bf16; the context reduction runs on
VectorE against a broadcast copy of w; softmax runs on VectorE/ScalarE.
"""

import numpy as np
import ml_dtypes

# ---------------------------------------------------------------------------
# problem constants (hardcoded per contract — no reading of spec.json)
B, T, E, A = 32, 2048, 1024, 512
N_CORES = 8
BPC = B // N_CORES            # batch rows per core
KF = 31                       # conv taps (2*15+1)
SCALING = 2.0
NEG = -1.0e30
PAD_T = T + 32                # padded att_prev row (15 left, 17 right)

_BF = ml_dtypes.bfloat16


# ---------------------------------------------------------------------------
def _apply_walrus_single_wait_patch():
    """The external walrus build in this container rejects any instruction
    carrying more than one sem-wait command.  Split multi-wait instructions by
    hoisting extra waits onto same-engine InstNoOp's."""
    import concourse.mybir as mybir
    import concourse.tile as tile
    from concourse.vector_clock import ScopedClock

    if getattr(tile.TileContext, "_single_wait_patched", False):
        return

    def _split_waits(tc, inst):
        si = getattr(inst, "sync_info", None)
        if si is None or inst.engine is None:
            return
        waits = list(si.on_wait)
        if len(waits) <= 1:
            return
        for w in waits[:-1]:
            nop = mybir.InstNoOp(
                name=tc.nc.get_next_instruction_name(),
                engine=inst.engine,
                sync_info=mybir.SyncInfo(on_wait=[w], on_update=[]),
                bass_nofuse=True,
            )
            tc._commit_instruction(nop)
        si.on_wait = waits[-1:]

    _orig_cl = tile.TileContext._commit_and_lower

    def _commit_and_lower(self, inst, original_block, old_bb_map, bb_to_exit_bb):
        _split_waits(self, inst)
        return _orig_cl(self, inst, original_block, old_bb_map, bb_to_exit_bb)

    def _drain_and_barrier(self, tick_clock, wait_clock):
        drain_inst = self.nc.sync.drain()
        wait_clock.add_sem_waits(
            drain_inst.ins, ScopedClock({None: tick_clock.global_clock})
        )
        si = drain_inst.ins.sync_info
        waits = list(si.on_wait)
        if len(waits) > 1:
            si.on_wait = waits[:1]
            for w in waits[1:]:
                nop = self.nc.sync.nop(nofuse=True, hint="drain_wait_spill")
                nop.ins.sync_info = mybir.SyncInfo(on_wait=[w], on_update=[])
        self.nc.all_engine_barrier()
        assert self.sems is not None
        popped = self.nc._tile_sem_poison_stack.pop()
        assert popped is self._sem_poison
        self.nc.clear_and_free_semaphores(list(self.sems.allocated().values()))
        self.nc.all_engine_barrier()

    tile.TileContext._commit_and_lower = _commit_and_lower
    tile.TileContext._drain_and_barrier = _drain_and_barrier
    tile.TileContext._single_wait_patched = True


# ---------------------------------------------------------------------------
def _build_program(repeat=1, phase="full"):
    import concourse.bass as bass
    import concourse.mybir as mybir
    import concourse.tile as tile

    _apply_walrus_single_wait_patch()

    F = mybir.dt.float32
    BF = mybir.dt.bfloat16
    AF = mybir.ActivationFunctionType
    ALU = mybir.AluOpType
    AX = mybir.AxisListType

    nc = bass.Bass()
    p_encT = nc.declare_dram_parameter("encT", [BPC, 128, 8, T], BF, isOutput=False)
    p_encN = nc.declare_dram_parameter("encN", [BPC, 128, 16, E], BF, isOutput=False)
    p_appad = nc.declare_dram_parameter("appad", [BPC, PAD_T], BF, isOutput=False)
    p_wenc = nc.declare_dram_parameter("wenc", [128, 8, A], BF, isOutput=False)
    p_ms = nc.declare_dram_parameter("msmall", [32, A], BF, isOutput=False)
    p_wg = nc.declare_dram_parameter("wg2", [128, 4], BF, isOutput=False)
    p_bi = nc.declare_dram_parameter("biasc", [128, BPC * 4], F, isOutput=False)
    p_nm = nc.declare_dram_parameter("negm", [BPC, T], F, isOutput=False)
    p_wout = nc.declare_dram_parameter("w_out", [BPC, T], F, isOutput=True)
    p_cout = nc.declare_dram_parameter("c_out", [BPC, E], F, isOutput=True)

    with tile.TileContext(nc) as tc:
        with (
            tc.tile_pool(name="const", bufs=1) as const,
            tc.tile_pool(name="et", bufs=2) as et_pool,
            tc.tile_pool(name="th", bufs=2) as th_pool,
            tc.tile_pool(name="win", bufs=2) as win_pool,
            tc.tile_pool(name="rows", bufs=2) as row_pool,
            tc.tile_pool(name="en", bufs=1) as en_pool,
            tc.tile_pool(name="ct", bufs=2) as ct_pool,
            tc.tile_pool(name="ppool", bufs=4, space="PSUM") as ppool,
            tc.tile_pool(name="epool", bufs=2, space="PSUM") as epool,
            tc.tile_pool(name="cpool", bufs=2, space="PSUM") as cpool,
            tc.tile_pool(name="dram", bufs=1, space="DRAM") as dram,
        ):
            WE = const.tile([128, 8, A], BF, tag="WE")
            ONE1 = const.tile([1, 1], F, tag="ONE1")
            nc.vector.memset(ONE1[:], 1.0)
            MS4 = const.tile([128, A], BF, tag="MS4")
            WG = const.tile([128, 4], BF, tag="WG")
            BI = const.tile([128, BPC * 4], F, tag="BI")
            nc.sync.dma_start(WE[:], p_wenc[:, :, :])
            for g in range(4):
                nc.sync.dma_start(MS4[32 * g : 32 * g + 32, :], p_ms[:, :])
            nc.sync.dma_start(WG[:], p_wg[:, :])
            nc.sync.dma_start(BI[:], p_bi[:, :])

            iters = [bb % BPC for bb in range(BPC * repeat)]

            def emit_loads(b):
                """Queue batch b's input DMAs. Big streams go on the ACT
                HWDGE ring, small ones on the SP ring."""
                ET = et_pool.tile([128, 8, T], BF, tag="ET")
                for k in range(4):
                    nc.sync.dma_start(ET[:, 2 * k : 2 * k + 2, :], p_encT[b, :, 2 * k : 2 * k + 2, :])
                WIN4 = win_pool.tile([128, T], BF, tag="WIN4")
                for g in range(4):
                    nc.sync.dma_start(
                        WIN4[32 * g : 32 * g + 32, :],
                        bass.AP(p_appad, b * PAD_T, [[1, 32], [1, T]]),
                    )
                nm = row_pool.tile([1, T], F, tag="nm")
                nc.sync.dma_start(nm[0:1, :], p_nm[b : b + 1, :])
                EN = en_pool.tile([128, 16, E], BF, tag="EN")
                nc.gpsimd.dma_start(EN[:], p_encN[b, :, :, :])
                return WIN4, nm, ET, EN

            loads = emit_loads(iters[0])
            pending_ctx = None
            for bi, b in enumerate(iters):
                WIN4, nm, ET, EN = loads

                # ---- energies: k-major quarters; att starts each PSUM tile
                TH = th_pool.tile([128, 4, T], BF, tag="TH")
                for ih in range(2):
                    for jh in range(2):
                        ii = (2 * ih, 2 * ih + 1)
                        jj = (2 * jh, 2 * jh + 1)
                        Pq = {}
                        for i in ii:
                            for j in jj:
                                P = ppool.tile([128, 512], F, tag="P")
                                Pq[(i, j)] = P
                        for k in range(8):
                            for i in ii:
                                for j in jj:
                                    nc.tensor.matmul(
                                        Pq[(i, j)][:, :],
                                        WE[:, k, i * 128 : (i + 1) * 128],
                                        ET[:, k, j * 512 : (j + 1) * 512],
                                        start=(k == 0),
                                        stop=False,
                                    )
                        for i in ii:
                            for j in jj:
                                nc.tensor.matmul(
                                    Pq[(i, j)][:, :],
                                    MS4[32 * i : 32 * i + 32, i * 128 : (i + 1) * 128],
                                    WIN4[32 * i : 32 * i + 32, j * 512 : (j + 1) * 512],
                                    start=False,
                                    stop=True,
                                    tile_position=(32 * i, 0),
                                )
                        for i in ii:
                            for j in jj:
                                nc.scalar.activation(
                                    TH[:, i, j * 512 : (j + 1) * 512],
                                    Pq[(i, j)][:, :],
                                    AF.Tanh,
                                    bias=BI[:, b * 4 + i : b * 4 + i + 1],
                                    scale=1.0,
                                )

                # ---- queue next iteration's loads before any blocking op --
                if bi + 1 < len(iters):
                    loads = emit_loads(iters[bi + 1])
                if phase == "energy":
                    nc.gpsimd.dma_start(p_wout[b : b + 1, 0:512], TH[0:1, 0, 0:512])
                    continue

                # ---- e row: 2*W_g contraction over a; mask folded in ------
                e_sb = row_pool.tile([1, T], F, tag="e_sb")
                msc = row_pool.tile([1, 8], F, tag="msc")
                pex = row_pool.tile([1, T], F, tag="pex")
                for j in range(4):
                    EP = epool.tile([128, 512], F, tag="EP")
                    for i in range(4):
                        nc.tensor.matmul(
                            EP[0:1, :],
                            WG[:, i : i + 1],
                            TH[:, i, j * 512 : (j + 1) * 512],
                            start=(i == 0),
                            stop=(i == 3),
                        )
                    ej = e_sb[0:1, j * 512 : (j + 1) * 512]
                    nc.vector.tensor_tensor(
                        ej, nm[0:1, j * 512 : (j + 1) * 512], EP[0:1, :], op=ALU.add
                    )
                    nc.vector.tensor_reduce(
                        msc[0:1, 4 + j : 5 + j], ej, axis=AX.X, op=ALU.max
                    )

                # ---- masked softmax ---------------------------------------
                nc.vector.tensor_reduce(msc[0:1, 0:1], msc[0:1, 4:8], axis=AX.X, op=ALU.max)
                nc.scalar.mul(msc[0:1, 1:2], msc[0:1, 0:1], -1.0)
                nc.scalar.activation(
                    pex[0:1, :], e_sb[0:1, :], AF.Exp,
                    bias=msc[0:1, 1:2], scale=1.0, accum_out=msc[0:1, 2:3],
                )
                if phase == "noctx":
                    nc.vector.reciprocal(msc[0:1, 3:4], msc[0:1, 2:3])
                    nc.vector.tensor_scalar_mul(pex[0:1, :], pex[0:1, :], msc[0:1, 3:4])
                    nc.gpsimd.dma_start(p_wout[b : b + 1, :], pex[0:1, :])
                    continue

                # ---- context (deferred one iteration: runs under the next
                # batch's energy phase so PE never blocks on softmax) -------
                def emit_ctx(b, EN, pex):
                    WTP = cpool.tile([128, 512], F, tag="CP")
                    for t_i in range(16):
                        nc.tensor.matmul(
                            WTP[:, t_i : t_i + 1],
                            pex[0:1, 128 * t_i : 128 * (t_i + 1)],
                            ONE1[0:1, 0:1],
                            start=True,
                            stop=True,
                        )
                    wT = ct_pool.tile([128, 16], BF, tag="wT")
                    nc.vector.tensor_copy(wT[:, :], WTP[:, 0:16])
                    cS = ct_pool.tile([1, E], F, tag="cS")
                    for h in range(2):
                        CP = cpool.tile([128, 512], F, tag="CP")
                        for t_i in range(16):
                            g = t_i % 2
                            nc.tensor.matmul(
                                CP[32 * g : 32 * g + 1, :],
                                wT[:, t_i : t_i + 1],
                                EN[:, t_i, h * 512 : (h + 1) * 512],
                                start=(t_i < 2),
                                stop=(t_i >= 14),
                                tile_position=(0, 32 * g),
                            )
                        half = cS[0:1, h * 512 : (h + 1) * 512]
                        nc.vector.tensor_copy(half, CP[0:1, :])
                        nc.vector.tensor_tensor(half, half, CP[32:33, :], op=ALU.add)
                    nc.gpsimd.dma_start(p_cout[b : b + 1, :], cS[0:1, :])

                if pending_ctx is not None:
                    emit_ctx(*pending_ctx)
                pending_ctx = (b, EN, pex)

                # softmax tail after the deferred ctx so the DVE stream
                # doesn't block the previous batch's context ops
                nc.vector.reciprocal(msc[0:1, 3:4], msc[0:1, 2:3])
                nc.vector.tensor_scalar_mul(pex[0:1, :], pex[0:1, :], msc[0:1, 3:4])
                nc.gpsimd.dma_start(p_wout[b : b + 1, :], pex[0:1, :])

            if pending_ctx is not None:
                emit_ctx(*pending_ctx)

    return nc


# ---------------------------------------------------------------------------
def kernel(enc_hs_pad, enc_hs_len, dec_z, att_prev,
           W_enc, b_enc, W_dec, W_att, conv_w, W_g, b_g):
    from concourse.bass_utils import run_bass_kernel_spmd

    enc = np.ascontiguousarray(np.asarray(enc_hs_pad, dtype=np.float32))
    lens = np.asarray(enc_hs_len).astype(np.int64)
    dec_z = np.asarray(dec_z, dtype=np.float32)
    att_prev = np.asarray(att_prev, dtype=np.float32)
    W_enc = np.asarray(W_enc, dtype=np.float32)
    b_enc = np.asarray(b_enc, dtype=np.float32)
    W_dec = np.asarray(W_dec, dtype=np.float32)
    W_att = np.asarray(W_att, dtype=np.float32)
    conv_w = np.asarray(conv_w, dtype=np.float32)
    W_g = np.asarray(W_g, dtype=np.float32)
    b_g = np.asarray(b_g, dtype=np.float32)

    # ---- host-side prep (tiny reshapes/folds) -----------------------------
    # enc transposed: [b, p, c, t] = enc[b, t, c*128 + p]
    encT = enc.reshape(B, T, 8, 128).transpose(0, 3, 2, 1).astype(_BF)
    encN = enc.reshape(B, 16, 128, E).transpose(0, 2, 1, 3).astype(_BF)
    appad = np.zeros((B, PAD_T), dtype=_BF)
    appad[:, 15 : 15 + T] = att_prev.astype(_BF)
    # wenc: [p, k, a] = W_enc[k*128 + p, a]
    wenc = W_enc.reshape(8, 128, A).transpose(1, 0, 2).astype(_BF)
    # folded location-conv: att_c[t, a] = sum_k M[k, a] * appad[t + k]
    Mfold = conv_w[:, 0, :].T @ W_att            # (31, A)
    msmall = np.zeros((32, A), dtype=_BF)
    msmall[:KF] = Mfold.astype(_BF)
    # wg2: [p, i] = SCALING * W_g[i*128 + p]
    wg2 = (SCALING * W_g[:, 0]).reshape(4, 128).T.astype(_BF)
    wg2 = np.ascontiguousarray(wg2)
    # per-partition tanh bias: dec proj + b_enc
    bias = dec_z @ W_dec + b_enc                 # (B, A)
    # neg mask rows (scaled b_g folded in): 2*b_g where valid else -1e30
    tt = np.arange(T)[None, :]
    negm = np.where(tt < lens[:, None], SCALING * b_g[0], NEG).astype(np.float32)

    nc = _build_program()

    in_maps = []
    for i in range(N_CORES):
        sl = slice(i * BPC, (i + 1) * BPC)
        bias_core = (
            bias[sl].reshape(BPC, 4, 128).transpose(2, 0, 1).reshape(128, BPC * 4)
        )
        in_maps.append({
            "encT": np.ascontiguousarray(encT[sl]),
            "encN": np.ascontiguousarray(encN[sl]),
            "appad": np.ascontiguousarray(appad[sl]),
            "wenc": np.ascontiguousarray(wenc),
            "msmall": msmall,
            "wg2": wg2,
            "biasc": np.ascontiguousarray(bias_core),
            "negm": np.ascontiguousarray(negm[sl]),
        })

    res = run_bass_kernel_spmd(nc, in_maps, list(range(N_CORES)))

    w_full = np.concatenate([res.results[i]["w_out"] for i in range(N_CORES)], axis=0)
    c_full = np.concatenate(
        [res.results[i]["c_out"] for i in range(N_CORES)], axis=0
    )
    return c_full.astype(np.float32), w_full.astype(np.float32)


# revision 18
# speedup vs baseline: 1.5439x; 1.4129x over previous
"""Trainium2 Bass kernel for location-aware attention (AttLoc).

Computation (per batch row b):
  pre_enc = enc @ W_enc + b_enc                  (T, A)
  att_c   = conv1d(att_prev) @ W_att             (T, A)   [folded: M = conv_w.T @ W_att]
  dec_t   = dec_z @ W_dec                        (A,)     [host-computed, per-partition bias]
  e       = tanh(pre_enc + att_c + dec_t) @ W_g + b_g
  w       = softmax(2 * e, masked t >= len)
  c       = sum_t w[t] * enc[t, :]

Distribution: data-parallel over batch B=32 across 8 NeuronCores (4 rows each).
Device layouts (host-prepared): enc in both [e-partition, t-free] ("encT", for
the energy matmul, which contracts over e) and [t-partition, e-free] ("encN",
for the context reduction, which contracts over t), both bf16.  The energy
matmul and both reductions run on TensorE (context/W_g as packed M=1 matmuls);
tanh/exp on ScalarE with fused per-partition bias; softmax row ops on VectorE.
Each batch's context phase is deferred one iteration so it executes under the
next batch's energy matmuls; input DMAs issue from wait-free rings (SP for the
sequential enc stream, GPSIMD for the rest) so prefetch is never blocked by
compute waits.
"""

import numpy as np
import ml_dtypes

# ---------------------------------------------------------------------------
# problem constants (hardcoded per contract — no reading of spec.json)
B, T, E, A = 32, 2048, 1024, 512
N_CORES = 8
BPC = B // N_CORES            # batch rows per core
KF = 31                       # conv taps (2*15+1)
SCALING = 2.0
NEG = -1.0e30
PAD_T = T + 32                # padded att_prev row (15 left, 17 right)

_BF = ml_dtypes.bfloat16


# ---------------------------------------------------------------------------
def _apply_walrus_single_wait_patch():
    """The external walrus build in this container rejects any instruction
    carrying more than one sem-wait command.  Split multi-wait instructions by
    hoisting extra waits onto same-engine InstNoOp's."""
    import concourse.mybir as mybir
    import concourse.tile as tile
    from concourse.vector_clock import ScopedClock

    if getattr(tile.TileContext, "_single_wait_patched", False):
        return

    def _split_waits(tc, inst):
        si = getattr(inst, "sync_info", None)
        if si is None or inst.engine is None:
            return
        waits = list(si.on_wait)
        if len(waits) <= 1:
            return
        for w in waits[:-1]:
            nop = mybir.InstNoOp(
                name=tc.nc.get_next_instruction_name(),
                engine=inst.engine,
                sync_info=mybir.SyncInfo(on_wait=[w], on_update=[]),
                bass_nofuse=True,
            )
            tc._commit_instruction(nop)
        si.on_wait = waits[-1:]

    _orig_cl = tile.TileContext._commit_and_lower

    def _commit_and_lower(self, inst, original_block, old_bb_map, bb_to_exit_bb):
        _split_waits(self, inst)
        return _orig_cl(self, inst, original_block, old_bb_map, bb_to_exit_bb)

    def _drain_and_barrier(self, tick_clock, wait_clock):
        drain_inst = self.nc.sync.drain()
        wait_clock.add_sem_waits(
            drain_inst.ins, ScopedClock({None: tick_clock.global_clock})
        )
        si = drain_inst.ins.sync_info
        waits = list(si.on_wait)
        if len(waits) > 1:
            si.on_wait = waits[:1]
            for w in waits[1:]:
                nop = self.nc.sync.nop(nofuse=True, hint="drain_wait_spill")
                nop.ins.sync_info = mybir.SyncInfo(on_wait=[w], on_update=[])
        self.nc.all_engine_barrier()
        assert self.sems is not None
        popped = self.nc._tile_sem_poison_stack.pop()
        assert popped is self._sem_poison
        self.nc.clear_and_free_semaphores(list(self.sems.allocated().values()))
        self.nc.all_engine_barrier()

    tile.TileContext._commit_and_lower = _commit_and_lower
    tile.TileContext._drain_and_barrier = _drain_and_barrier
    tile.TileContext._single_wait_patched = True


# ---------------------------------------------------------------------------
def _build_program(repeat=1, phase="full"):
    import concourse.bass as bass
    import concourse.mybir as mybir
    import concourse.tile as tile

    _apply_walrus_single_wait_patch()

    F = mybir.dt.float32
    BF = mybir.dt.bfloat16
    AF = mybir.ActivationFunctionType
    ALU = mybir.AluOpType
    AX = mybir.AxisListType

    nc = bass.Bass()
    p_encT = nc.declare_dram_parameter("encT", [BPC, 128, 8, T], BF, isOutput=False)
    p_encN = nc.declare_dram_parameter("encN", [BPC, 128, 16, E], BF, isOutput=False)
    p_appad = nc.declare_dram_parameter("appad", [BPC, PAD_T], BF, isOutput=False)
    p_wenc = nc.declare_dram_parameter("wenc", [128, 8, A], BF, isOutput=False)
    p_ms = nc.declare_dram_parameter("msmall", [32, A], BF, isOutput=False)
    p_wg = nc.declare_dram_parameter("wg2", [128, 4], BF, isOutput=False)
    p_bi = nc.declare_dram_parameter("biasc", [128, BPC * 4], F, isOutput=False)
    p_nm = nc.declare_dram_parameter("negm", [BPC, T], F, isOutput=False)
    p_wout = nc.declare_dram_parameter("w_out", [BPC, T], F, isOutput=True)
    p_cout = nc.declare_dram_parameter("c_out", [BPC, E], F, isOutput=True)

    with tile.TileContext(nc) as tc:
        with (
            tc.tile_pool(name="const", bufs=1) as const,
            tc.tile_pool(name="et", bufs=2) as et_pool,
            tc.tile_pool(name="th", bufs=2) as th_pool,
            tc.tile_pool(name="win", bufs=2) as win_pool,
            tc.tile_pool(name="rows", bufs=2) as row_pool,
            tc.tile_pool(name="en", bufs=1) as en_pool,
            tc.tile_pool(name="ct", bufs=2) as ct_pool,
            tc.tile_pool(name="ppool", bufs=4, space="PSUM") as ppool,
            tc.tile_pool(name="epool", bufs=2, space="PSUM") as epool,
            tc.tile_pool(name="cpool", bufs=2, space="PSUM") as cpool,
            tc.tile_pool(name="dram", bufs=1, space="DRAM") as dram,
        ):
            WE = const.tile([128, 8, A], BF, tag="WE")
            ONE1 = const.tile([1, 1], F, tag="ONE1")
            nc.vector.memset(ONE1[:], 1.0)
            MS4 = const.tile([128, A], BF, tag="MS4")
            WG = const.tile([128, 4], BF, tag="WG")
            BI = const.tile([128, BPC * 4], F, tag="BI")
            nc.sync.dma_start(WE[:], p_wenc[:, :, :])
            for g in range(4):
                nc.sync.dma_start(MS4[32 * g : 32 * g + 32, :], p_ms[:, :])
            nc.sync.dma_start(WG[:], p_wg[:, :])
            nc.sync.dma_start(BI[:], p_bi[:, :])

            iters = [bb % BPC for bb in range(BPC * repeat)]

            def emit_loads(b):
                """Queue batch b's input DMAs. Big streams go on the ACT
                HWDGE ring, small ones on the SP ring."""
                ET = et_pool.tile([128, 8, T], BF, tag="ET")
                for k in range(4):
                    nc.sync.dma_start(ET[:, 2 * k : 2 * k + 2, :], p_encT[b, :, 2 * k : 2 * k + 2, :])
                WIN4 = win_pool.tile([128, T], BF, tag="WIN4")
                for g in range(4):
                    nc.sync.dma_start(
                        WIN4[32 * g : 32 * g + 32, :],
                        bass.AP(p_appad, b * PAD_T, [[1, 32], [1, T]]),
                    )
                nm = row_pool.tile([1, T], F, tag="nm")
                nc.sync.dma_start(nm[0:1, :], p_nm[b : b + 1, :])
                EN = en_pool.tile([128, 16, E], BF, tag="EN")
                nc.gpsimd.dma_start(EN[:], p_encN[b, :, :, :])
                return WIN4, nm, ET, EN

            loads = emit_loads(iters[0])
            pending_ctx = None
            for bi, b in enumerate(iters):
                WIN4, nm, ET, EN = loads

                # ---- energies: k-major quarters; att starts each PSUM tile
                TH = th_pool.tile([128, 4, T], BF, tag="TH")
                for ih in range(2):
                    for jh in range(2):
                        ii = (2 * ih, 2 * ih + 1)
                        jj = (2 * jh, 2 * jh + 1)
                        Pq = {}
                        for i in ii:
                            for j in jj:
                                P = ppool.tile([128, 512], F, tag="P")
                                Pq[(i, j)] = P
                        for k in range(8):
                            for i in ii:
                                for j in jj:
                                    nc.tensor.matmul(
                                        Pq[(i, j)][:, :],
                                        WE[:, k, i * 128 : (i + 1) * 128],
                                        ET[:, k, j * 512 : (j + 1) * 512],
                                        start=(k == 0),
                                        stop=False,
                                    )
                        for i in ii:
                            for j in jj:
                                nc.tensor.matmul(
                                    Pq[(i, j)][:, :],
                                    MS4[32 * i : 32 * i + 32, i * 128 : (i + 1) * 128],
                                    WIN4[32 * i : 32 * i + 32, j * 512 : (j + 1) * 512],
                                    start=False,
                                    stop=True,
                                    tile_position=(32 * i, 0),
                                )
                        for i in ii:
                            for j in jj:
                                nc.scalar.activation(
                                    TH[:, i, j * 512 : (j + 1) * 512],
                                    Pq[(i, j)][:, :],
                                    AF.Tanh,
                                    bias=BI[:, b * 4 + i : b * 4 + i + 1],
                                    scale=1.0,
                                )

                # ---- queue next iteration's loads before any blocking op --
                if bi + 1 < len(iters):
                    loads = emit_loads(iters[bi + 1])
                if phase == "energy":
                    nc.gpsimd.dma_start(p_wout[b : b + 1, 0:512], TH[0:1, 0, 0:512])
                    continue

                # ---- e row: 2*W_g contraction over a; mask folded in ------
                e_sb = row_pool.tile([1, T], F, tag="e_sb")
                msc = row_pool.tile([1, 8], F, tag="msc")
                pex = row_pool.tile([1, T], F, tag="pex")
                for j in range(4):
                    EP = epool.tile([128, 512], F, tag="EP")
                    for i in range(4):
                        nc.tensor.matmul(
                            EP[0:1, :],
                            WG[:, i : i + 1],
                            TH[:, i, j * 512 : (j + 1) * 512],
                            start=(i == 0),
                            stop=(i == 3),
                        )
                    ej = e_sb[0:1, j * 512 : (j + 1) * 512]
                    nc.vector.tensor_tensor(
                        ej, nm[0:1, j * 512 : (j + 1) * 512], EP[0:1, :], op=ALU.add
                    )
                    nc.vector.tensor_reduce(
                        msc[0:1, 4 + j : 5 + j], ej, axis=AX.X, op=ALU.max
                    )

                # ---- masked softmax ---------------------------------------
                nc.vector.tensor_reduce(msc[0:1, 0:1], msc[0:1, 4:8], axis=AX.X, op=ALU.max)
                nc.scalar.mul(msc[0:1, 1:2], msc[0:1, 0:1], -1.0)
                nc.scalar.activation(
                    pex[0:1, :], e_sb[0:1, :], AF.Exp,
                    bias=msc[0:1, 1:2], scale=1.0, accum_out=msc[0:1, 2:3],
                )
                if phase == "noctx":
                    nc.vector.reciprocal(msc[0:1, 3:4], msc[0:1, 2:3])
                    nc.vector.tensor_scalar_mul(pex[0:1, :], pex[0:1, :], msc[0:1, 3:4])
                    nc.gpsimd.dma_start(p_wout[b : b + 1, :], pex[0:1, :])
                    continue

                # ---- context (deferred one iteration: runs under the next
                # batch's energy phase so PE never blocks on softmax) -------
                def emit_ctx(b, EN, pex):
                    WTP = cpool.tile([128, 512], F, tag="CP")
                    for t_i in range(16):
                        nc.tensor.matmul(
                            WTP[:, t_i : t_i + 1],
                            pex[0:1, 128 * t_i : 128 * (t_i + 1)],
                            ONE1[0:1, 0:1],
                            start=True,
                            stop=True,
                        )
                    wT = ct_pool.tile([128, 16], BF, tag="wT")
                    nc.vector.tensor_copy(wT[:, :], WTP[:, 0:16])
                    cS = ct_pool.tile([1, E], F, tag="cS")
                    for h in range(2):
                        CP = cpool.tile([128, 512], F, tag="CP")
                        for t_i in range(16):
                            g = t_i % 2
                            nc.tensor.matmul(
                                CP[32 * g : 32 * g + 1, :],
                                wT[:, t_i : t_i + 1],
                                EN[:, t_i, h * 512 : (h + 1) * 512],
                                start=(t_i < 2),
                                stop=(t_i >= 14),
                                tile_position=(0, 32 * g),
                            )
                        half = cS[0:1, h * 512 : (h + 1) * 512]
                        nc.vector.tensor_copy(half, CP[0:1, :])
                        nc.vector.tensor_tensor(half, half, CP[32:33, :], op=ALU.add)
                    nc.gpsimd.dma_start(p_cout[b : b + 1, :], cS[0:1, :])

                if pending_ctx is not None:
                    emit_ctx(*pending_ctx)
                pending_ctx = (b, EN, pex)

                # softmax tail after the deferred ctx so the DVE stream
                # doesn't block the previous batch's context ops
                nc.vector.reciprocal(msc[0:1, 3:4], msc[0:1, 2:3])
                nc.vector.tensor_scalar_mul(pex[0:1, :], pex[0:1, :], msc[0:1, 3:4])
                nc.gpsimd.dma_start(p_wout[b : b + 1, :], pex[0:1, :])

            if pending_ctx is not None:
                emit_ctx(*pending_ctx)

    return nc


# ---------------------------------------------------------------------------
def kernel(enc_hs_pad, enc_hs_len, dec_z, att_prev,
           W_enc, b_enc, W_dec, W_att, conv_w, W_g, b_g):
    from concourse.bass_utils import run_bass_kernel_spmd

    enc = np.ascontiguousarray(np.asarray(enc_hs_pad, dtype=np.float32))
    lens = np.asarray(enc_hs_len).astype(np.int64)
    dec_z = np.asarray(dec_z, dtype=np.float32)
    att_prev = np.asarray(att_prev, dtype=np.float32)
    W_enc = np.asarray(W_enc, dtype=np.float32)
    b_enc = np.asarray(b_enc, dtype=np.float32)
    W_dec = np.asarray(W_dec, dtype=np.float32)
    W_att = np.asarray(W_att, dtype=np.float32)
    conv_w = np.asarray(conv_w, dtype=np.float32)
    W_g = np.asarray(W_g, dtype=np.float32)
    b_g = np.asarray(b_g, dtype=np.float32)

    # ---- host-side prep (tiny reshapes/folds) -----------------------------
    # enc transposed: [b, p, c, t] = enc[b, t, c*128 + p]
    encT = enc.reshape(B, T, 8, 128).transpose(0, 3, 2, 1).astype(_BF)
    encN = enc.reshape(B, 16, 128, E).transpose(0, 2, 1, 3).astype(_BF)
    appad = np.zeros((B, PAD_T), dtype=_BF)
    appad[:, 15 : 15 + T] = att_prev.astype(_BF)
    # wenc: [p, k, a] = W_enc[k*128 + p, a]
    wenc = W_enc.reshape(8, 128, A).transpose(1, 0, 2).astype(_BF)
    # folded location-conv: att_c[t, a] = sum_k M[k, a] * appad[t + k]
    Mfold = conv_w[:, 0, :].T @ W_att            # (31, A)
    msmall = np.zeros((32, A), dtype=_BF)
    msmall[:KF] = Mfold.astype(_BF)
    # wg2: [p, i] = SCALING * W_g[i*128 + p]
    wg2 = (SCALING * W_g[:, 0]).reshape(4, 128).T.astype(_BF)
    wg2 = np.ascontiguousarray(wg2)
    # per-partition tanh bias: dec proj + b_enc
    bias = dec_z @ W_dec + b_enc                 # (B, A)
    # neg mask rows (scaled b_g folded in): 2*b_g where valid else -1e30
    tt = np.arange(T)[None, :]
    negm = np.where(tt < lens[:, None], SCALING * b_g[0], NEG).astype(np.float32)

    nc = _build_program()

    in_maps = []
    for i in range(N_CORES):
        sl = slice(i * BPC, (i + 1) * BPC)
        bias_core = (
            bias[sl].reshape(BPC, 4, 128).transpose(2, 0, 1).reshape(128, BPC * 4)
        )
        in_maps.append({
            "encT": np.ascontiguousarray(encT[sl]),
            "encN": np.ascontiguousarray(encN[sl]),
            "appad": np.ascontiguousarray(appad[sl]),
            "wenc": np.ascontiguousarray(wenc),
            "msmall": msmall,
            "wg2": wg2,
            "biasc": np.ascontiguousarray(bias_core),
            "negm": np.ascontiguousarray(negm[sl]),
        })

    res = run_bass_kernel_spmd(nc, in_maps, list(range(N_CORES)))

    w_full = np.concatenate([res.results[i]["w_out"] for i in range(N_CORES)], axis=0)
    c_full = np.concatenate(
        [res.results[i]["c_out"] for i in range(N_CORES)], axis=0
    )
    return c_full.astype(np.float32), w_full.astype(np.float32)
